# revision 1
# baseline (speedup 1.0000x reference)
"""GCN layer (2x gcn_conv with GELU) on 8 Trainium2 NeuronCores.

Contract: kernel(**inputs) takes the FULL inputs of reference.setup_inputs()
and returns the FULL [100000, 64] float32 output.

Strategy (graph/data parallel, sharded by destination node):
- Edges partitioned by dst across 8 cores (12500 dst nodes each).
- Per core, edges sorted by (dst-group of 128, src-window of 32768, src).
- Layer 1 is "commuted": aggregate raw x rows (gathered per-edge via
  dma_gather), with the GCN norm coefficient dinv[src]*dinv[dst] folded into
  a one-hot selection matrix S; aggregate = S^T @ gathered via TensorE into
  PSUM; then the dense transform (W1, GELU, W2) runs per 128-dst group.
- g2 = dinv * (z1 @ W2) is exchanged via 13 chunked AllGathers (overlapped
  with compute) into Shared DRAM, then layer 2 aggregates g2 the same way.
"""
import sys
sys.path.insert(0, "/opt/trn_rl_repo")

import numpy as np
import os

PHASE = int(os.environ.get("GCN_PHASE", "3"))  # 1=L1 only, 2=+AG, 3=full

N = 100000
FIN = 128
FOUT = 64
NC = 8
RS = N // NC            # 12500 dst rows per core
GSZ = 128               # dst group size
GP = (RS + GSZ - 1) // GSZ   # 98 groups per core (last has 84 nodes)
WIN = 32768             # src index window (int16 limit)
NW = (N + WIN - 1) // WIN    # 4 windows
B = 12                  # dst groups per batch (PSUM residency)
CAP = 4096              # max tokens per dma_gather (16-bit sem wait limit)
AGL = 1024              # g2 rows per core per AllGather chunk
KAG = (RS + AGL - 1) // AGL  # 13 chunks (last 212 rows)
GPC = AGL // GSZ        # dst groups per AG chunk
CPW = WIN // (NC * AGL) # AG chunks per window

def _set_config(**kw):
    """Override module constants (for scaled-down tests) and derived values."""
    g = globals()
    g.update(kw)
    g["RS"] = g["N"] // g["NC"]
    g["GP"] = (g["RS"] + g["GSZ"] - 1) // g["GSZ"]
    g["NW"] = (g["N"] + g["WIN"] - 1) // g["WIN"]
    g["KAG"] = (g["RS"] + g["AGL"] - 1) // g["AGL"]
    g["GPC"] = g["AGL"] // g["GSZ"]
    g["CPW"] = g["WIN"] // (g["NC"] * g["AGL"])
    assert g["WIN"] % (g["NC"] * g["AGL"]) == 0
    assert g["AGL"] % g["GSZ"] == 0
    _cache.clear()

_cache = {}


# ----------------------------------------------------------------- host side

def _flat_g2_row(src):
    """Node id -> row in the window-split allgather layout.

    g2 flat layout: chunk k holds, core-major, each core's g2 rows
    [AGL*k, AGL*k+len_k): flat = 8*AGL*k + c*len_k + off.
    """
    c = src // RS
    r = src % RS
    k = r // AGL
    off = r - k * AGL
    len_k = np.minimum(RS - k * AGL, AGL)
    return NC * AGL * k + c * len_k + off


def _build_layer(rows_by_core, gl_by_core, dl_by_core, coef_by_core):
    """Shared program structure + per-core token data for one layer."""
    has_coef = coef_by_core is not None
    per_core = []
    cell_cnt = np.zeros((NC, GP, NW), dtype=np.int64)
    for c in range(NC):
        rows, gl, dl = rows_by_core[c], gl_by_core[c], dl_by_core[c]
        win = rows // WIN
        order = np.lexsort((rows, win, gl))
        rows, gl, dl, win = rows[order], gl[order], dl[order], win[order]
        coef = coef_by_core[c][order] if has_coef else None
        np.add.at(cell_cnt[c], (gl, win), 1)
        flat_sizes = cell_cnt[c].reshape(-1)
        starts = np.concatenate([[0], np.cumsum(flat_sizes)[:-1]]).reshape(GP, NW)
        per_core.append(dict(rows=rows, dl=dl.astype(np.float32), coef=coef,
                             starts=starts))

    cell_max = cell_cnt.max(axis=0)
    cell_pad = ((cell_max + 15) // 16) * 16
    empty = cell_pad.sum(axis=1) == 0
    cell_pad[empty, 0] = 16

    batches = [list(range(b, min(b + B, GP))) for b in range(0, GP, B)]
    calls = []
    mdescs = []      # [call_i, col, g, m_index, start, stop]
    total_cols_idx = 0

    def close_call(bi, w, gext, pos):
        nonlocal total_cols_idx
        ntok = ((pos + 127) // 128) * 128
        if ntok == 0:
            return
        ci = len(calls)
        calls.append(dict(batch=bi, w=w, ntok=ntok, gext=dict(gext),
                          idx_col0=total_cols_idx))
        total_cols_idx += ntok // 16
        for j in range(ntok // 128):
            lo, hi = j * 128, (j + 1) * 128
            for g, (s, e) in gext.items():
                if s < hi and e > lo:
                    mdescs.append([ci, j, g, len(mdescs), False, False])

    for bi, groups in enumerate(batches):
        for w in range(NW):
            gext = {}
            pos = 0
            for g in groups:
                sz = int(cell_pad[g, w])
                if not sz:
                    continue
                assert sz <= CAP, f"cell {g},{w} = {sz} exceeds CAP"
                if pos + sz > CAP:
                    close_call(bi, w, gext, pos)
                    gext, pos = {}, 0
                gext[g] = (pos, pos + sz)
                pos += sz
            close_call(bi, w, gext, pos)
    seen_first = set()
    last_of = {}
    first_call_of = {}
    for m in mdescs:
        key = (m[0], m[2])          # (call, group)
        if key not in seen_first:
            m[4] = True
            seen_first.add(key)
            bkey = (calls[m[0]]["batch"], m[2])
            if bkey not in first_call_of:
                first_call_of[bkey] = m[0]
        last_of[key] = m
    for m in last_of.values():
        m[5] = True
    M = len(mdescs)

    idx_arr = np.zeros((NC, 16, total_cols_idx), dtype=np.int16)
    dst_arr = np.full((NC, M, 128), -1.0, dtype=np.float32)
    coef_arr = np.zeros((NC, M, 128), dtype=np.float32) if has_coef else None

    for c in range(NC):
        pc = per_core[c]
        for call in calls:
            w = call["w"]
            ntok = call["ntok"]
            stream_rows = np.zeros(ntok, dtype=np.int64)
            for g, (s, e) in call["gext"].items():
                cnt = int(cell_cnt[c, g, w])
                st = pc["starts"][g, w]
                if cnt:
                    stream_rows[s:s + cnt] = pc["rows"][st:st + cnt] - w * WIN
                    stream_rows[s + cnt:e] = stream_rows[s + cnt - 1]
            c0 = call["idx_col0"]
            idx_arr[c, :, c0:c0 + ntok // 16] = (
                stream_rows.astype(np.int16).reshape(-1, 16).T)
        for ci, j, g, mi, _, _ in mdescs:
            call = calls[ci]
            w = call["w"]
            s, e = call["gext"][g]
            lo, hi = j * 128, (j + 1) * 128
            a = max(s, lo)
            cnt = int(cell_cnt[c, g, w])
            st = pc["starts"][g, w]
            real_hi = min(hi, s + cnt)
            if real_hi > a:
                k0, k1 = a - s, real_hi - s
                dst_arr[c, mi, a - lo:real_hi - lo] = pc["dl"][st + k0:st + k1]
                if has_coef:
                    coef_arr[c, mi, a - lo:real_hi - lo] = \
                        pc["coef"][st + k0:st + k1]

    out = dict(calls=calls, mdescs=mdescs, M=M, first_call_of=first_call_of,
               idx=np.tile(idx_arr, (1, 8, 1)),
               dst=np.ascontiguousarray(dst_arr.transpose(0, 2, 1)),
               total_idx_cols=total_cols_idx,
               batches=batches,
               max_ntok=max(c_["ntok"] for c_ in calls))
    if has_coef:
        out["coef"] = np.ascontiguousarray(coef_arr.transpose(0, 2, 1))
    return out


def _preprocess(x, edge_index, W1, b1, W2, b2):
    src = np.asarray(edge_index[0], dtype=np.int64)
    dst = np.asarray(edge_index[1], dtype=np.int64)
    deg = np.bincount(dst, minlength=N).astype(np.float32) + 1.0
    dinv = (1.0 / np.sqrt(deg)).astype(np.float32)

    core = dst // RS
    l1 = dict(rows=[], gl=[], dl=[], coef=[])
    l2 = dict(rows=[], gl=[], dl=[])
    for c in range(NC):
        m = core == c
        s, d = src[m], dst[m]
        rl = d - c * RS
        l1["rows"].append(s)
        l1["gl"].append(rl // GSZ)
        l1["dl"].append(rl % GSZ)
        l1["coef"].append((dinv[s] * dinv[d]).astype(np.float32))
        l2["rows"].append(_flat_g2_row(s))
        l2["gl"].append(rl // GSZ)
        l2["dl"].append(rl % GSZ)

    L1 = _build_layer(l1["rows"], l1["gl"], l1["dl"], l1["coef"])
    L2 = _build_layer(l2["rows"], l2["gl"], l2["dl"], None)

    dinv_pc = np.zeros((NC, 128, GP), dtype=np.float32)
    dinv2_pc = np.zeros((NC, 128, GP), dtype=np.float32)
    xs_pc = np.zeros((NC, GP * GSZ, FIN), dtype=np.float32)
    for c in range(NC):
        dvp = np.zeros(GP * GSZ, dtype=np.float32)
        dvp[:RS] = dinv[c * RS:(c + 1) * RS]
        dinv_pc[c] = dvp.reshape(GP, GSZ).T
        dinv2_pc[c] = (dvp ** 2).reshape(GP, GSZ).T
        xs_pc[c, :RS] = x[c * RS:(c + 1) * RS]

    b1bc = np.tile(np.asarray(b1, np.float32)[None, :], (128, 1))
    b2bc = np.tile(np.asarray(b2, np.float32)[None, :], (128, 1))
    return dict(L1=L1, L2=L2, dinv_pc=dinv_pc, dinv2_pc=dinv2_pc, xs_pc=xs_pc,
                b1bc=b1bc, b2bc=b2bc,
                W1=np.ascontiguousarray(np.asarray(W1, np.float32)),
                W2=np.ascontiguousarray(np.asarray(W2, np.float32)))


# --------------------------------------------------------------- device side

def _build_nc(pp, act="gelu"):
    import concourse.bacc as bacc
    import concourse.tile as tile
    from concourse import mybir
    from concourse.masks import make_identity

    L1, L2 = pp["L1"], pp["L2"]
    nc = bacc.Bacc(num_devices=NC)
    f32 = mybir.dt.float32

    t_x = nc.dram_tensor("x", [N, FIN], f32, kind="ExternalInput")
    t_xs = nc.dram_tensor("xs", [GP * GSZ, FIN], f32, kind="ExternalInput")
    t_idx1 = nc.dram_tensor("idx1", [128, L1["total_idx_cols"]],
                            mybir.dt.int16, kind="ExternalInput")
    t_idx2 = nc.dram_tensor("idx2", [128, L2["total_idx_cols"]],
                            mybir.dt.int16, kind="ExternalInput")
    t_dst1 = nc.dram_tensor("dst1", [128, L1["M"]], f32, kind="ExternalInput")
    t_coef1 = nc.dram_tensor("coef1", [128, L1["M"]], f32, kind="ExternalInput")
    t_dst2 = nc.dram_tensor("dst2", [128, L2["M"]], f32, kind="ExternalInput")
    t_dinv = nc.dram_tensor("dinv_pc", [128, GP], f32, kind="ExternalInput")
    t_dinv2 = nc.dram_tensor("dinv2_pc", [128, GP], f32, kind="ExternalInput")
    t_w1 = nc.dram_tensor("W1", [FIN, FOUT], f32, kind="ExternalInput")
    t_w2 = nc.dram_tensor("W2", [FOUT, FOUT], f32, kind="ExternalInput")
    t_b1 = nc.dram_tensor("b1bc", [128, FOUT], f32, kind="ExternalInput")
    t_b2 = nc.dram_tensor("b2bc", [128, FOUT], f32, kind="ExternalInput")
    t_out = nc.dram_tensor("out", [RS, FOUT], f32, kind="ExternalOutput")

    lens = [min(AGL, RS - k * AGL) for k in range(KAG)]
    g2in = [nc.dram_tensor(f"g2in{k}", [lens[k], FOUT], f32, kind="Internal")
            for k in range(KAG)]
    winlen = [min(WIN, N - w * WIN) for w in range(NW)]
    g2win = [nc.dram_tensor(f"g2win{w}", [winlen[w], FOUT], f32,
                            kind="Internal", addr_space="Shared")
             for w in range(NW)]

    actf = {"gelu": mybir.ActivationFunctionType.Gelu,
            "tanh": mybir.ActivationFunctionType.Tanh}[act]

    with tile.TileContext(nc) as tc:
        with (
            tc.tile_pool(name="const", bufs=1) as cp,
            tc.tile_pool(name="persist", bufs=1) as pers,
        ):
            ident = cp.tile([128, 128], f32)
            make_identity(nc, ident[:])
            iota_i = cp.tile([128, 128], mybir.dt.int32)
            nc.gpsimd.iota(iota_i[:], pattern=[[1, 128]], base=0,
                           channel_multiplier=0)
            iota = cp.tile([128, 128], f32)
            nc.vector.tensor_copy(iota[:], iota_i[:])
            w1_t = cp.tile([FIN, FOUT], f32)
            w2_t = cp.tile([FOUT, FOUT], f32)
            b1_t = cp.tile([128, FOUT], f32)
            b2_t = cp.tile([128, FOUT], f32)
            dinv_t = cp.tile([128, GP], f32)
            dinv2_t = cp.tile([128, GP], f32)
            for tt, src_t in ((w1_t, t_w1), (w2_t, t_w2), (b1_t, t_b1),
                              (b2_t, t_b2), (dinv_t, t_dinv),
                              (dinv2_t, t_dinv2)):
                nc.sync.dma_start(tt[:], src_t[:, :])
            dst1_t = cp.tile([128, L1["M"]], f32)
            coef1_t = cp.tile([128, L1["M"]], f32)
            dst2_t = cp.tile([128, L2["M"]], f32)
            nc.sync.dma_start(dst1_t[:], t_dst1[:, :])
            nc.sync.dma_start(coef1_t[:], t_coef1[:, :])
            nc.sync.dma_start(dst2_t[:], t_dst2[:, :])
            g2loc = pers.tile([128, GP, FOUT], f32)

            def run_phase(L, t_idx, dst_t, coef_t, elem, src_spaces, post_fn,
                          tag):
                with (
                    tc.tile_pool(name=f"gat{tag}", bufs=2) as gp_,
                    tc.tile_pool(name=f"idx{tag}", bufs=2) as ip_,
                    tc.tile_pool(name=f"agg{tag}", bufs=5,
                                 space="PSUM") as ap_,
                    tc.tile_pool(name=f"acc{tag}", bufs=B + 2) as cp2_,
                    tc.tile_pool(name=f"post{tag}", bufs=3) as wp_,
                    tc.tile_pool(name=f"pp{tag}", bufs=2, space="PSUM") as pp_,
                ):
                    mi_by_call = {}
                    for m in L["mdescs"]:
                        mi_by_call.setdefault(m[0], []).append(m)
                    acc_of = {}
                    for ci, call in enumerate(L["calls"]):
                        ntok = call["ntok"]
                        ncols = ntok // 128
                        w = call["w"]
                        bi = call["batch"]
                        groups_b = L["batches"][bi]
                        gtile = gp_.tile([128, L["max_ntok"] // 128, elem],
                                         f32, tag=f"g{tag}")
                        idxt = ip_.tile([128, L["max_ntok"] // 16],
                                        mybir.dt.int16, tag=f"i{tag}")
                        c0 = call["idx_col0"]
                        nc.sync.dma_start(idxt[:, :ntok // 16],
                                          t_idx[:, c0:c0 + ntok // 16])
                        nc.gpsimd.dma_gather(
                            out_ap=gtile[:, :ncols, :],
                            in_ap=src_spaces[w],
                            idxs_ap=idxt[:, :ntok // 16],
                            num_idxs=ntok,
                            num_idxs_reg=ntok,
                            elem_size=elem,
                            single_packet=False,
                        )
                        psum_cg = {}
                        for _, j, g, mi, st, sp in mi_by_call.get(ci, []):
                            if g not in psum_cg:
                                psum_cg[g] = ap_.tile(
                                    [128, elem], f32, tag=f"a{tag}",
                                    name=f"aggp{tag}", space="PSUM")
                            S = wp_.tile([128, 128], f32, tag=f"S{tag}")
                            if coef_t is not None:
                                nc.vector.tensor_scalar(
                                    out=S[:], in0=iota[:],
                                    scalar1=dst_t[:, mi:mi + 1],
                                    scalar2=coef_t[:, mi:mi + 1],
                                    op0=mybir.AluOpType.is_equal,
                                    op1=mybir.AluOpType.mult)
                            else:
                                nc.vector.tensor_scalar(
                                    out=S[:], in0=iota[:],
                                    scalar1=dst_t[:, mi:mi + 1],
                                    scalar2=None,
                                    op0=mybir.AluOpType.is_equal)
                            nc.tensor.matmul(psum_cg[g][:], lhsT=S[:],
                                             rhs=gtile[:, j, :],
                                             start=st, stop=sp)
                            if sp:
                                # fold this call's partial into the SBUF acc
                                if L["first_call_of"][(bi, g)] == ci:
                                    acc_of[g] = cp2_.tile(
                                        [128, elem], f32, tag=f"c{tag}",
                                        name=f"acc{tag}")
                                    nc.vector.tensor_copy(acc_of[g][:],
                                                          psum_cg[g][:])
                                else:
                                    nc.vector.tensor_tensor(
                                        out=acc_of[g][:], in0=acc_of[g][:],
                                        in1=psum_cg[g][:],
                                        op=mybir.AluOpType.add)
                        is_last = (ci + 1 == len(L["calls"])
                                   or L["calls"][ci + 1]["batch"] != bi)
                        if is_last:
                            for g in groups_b:
                                post_fn(g, acc_of.pop(g), wp_, pp_)
                    assert not acc_of

            # ---------------- phase A: layer 1 + transform
            ag_emitted = [False] * KAG

            def post_l1(g, agg, wp_, pp_):
                xd = wp_.tile([128, FIN], f32, tag="xd")
                nc.sync.dma_start(xd[:], t_xs[g * GSZ:(g + 1) * GSZ, :])
                u = wp_.tile([128, FIN], f32, tag="u")
                nc.vector.tensor_scalar(out=u[:], in0=xd[:],
                                        scalar1=dinv2_t[:, g:g + 1],
                                        scalar2=None,
                                        op0=mybir.AluOpType.mult)
                v = wp_.tile([128, FIN], f32, tag="v")
                nc.vector.tensor_tensor(out=v[:], in0=u[:], in1=agg[:],
                                        op=mybir.AluOpType.add)
                bank = pp_.tile([128, 512], f32, tag="pb", space="PSUM")
                vT_p = bank[:, 0:128]
                h1_p = bank[:, 128:128 + FOUT]
                z1T_p = bank[:64, 192:320]
                h2_p = bank[:, 320:320 + FOUT]
                nc.tensor.transpose(vT_p, v[:], ident[:])
                vT = wp_.tile([128, 128], f32, tag="vTs")
                nc.vector.tensor_copy(vT[:], vT_p)
                nc.tensor.matmul(h1_p, lhsT=vT[:], rhs=w1_t[:], start=True,
                                 stop=True)
                h1b = wp_.tile([128, FOUT], f32, tag="h1b")
                nc.vector.tensor_tensor(out=h1b[:], in0=h1_p, in1=b1_t[:],
                                        op=mybir.AluOpType.add)
                z1 = wp_.tile([128, FOUT], f32, tag="z1")
                nc.scalar.activation(z1[:], h1b[:], actf)
                nc.tensor.transpose(z1T_p, z1[:], ident[:])
                z1T = wp_.tile([FOUT, 128], f32, tag="z1Ts")
                nc.vector.tensor_copy(z1T[:], z1T_p)
                nc.tensor.matmul(h2_p, lhsT=z1T[:], rhs=w2_t[:], start=True,
                                 stop=True)
                nc.vector.tensor_scalar(out=g2loc[:, g, :], in0=h2_p,
                                        scalar1=dinv_t[:, g:g + 1],
                                        scalar2=None,
                                        op0=mybir.AluOpType.mult)
                k = g // GPC
                r0 = (g % GPC) * GSZ
                nrow = min(GSZ, lens[k] - r0)
                nc.sync.dma_start(g2in[k][r0:r0 + nrow, :],
                                  g2loc[:nrow, g, :])
                if PHASE >= 3:
                    pass
                elif PHASE == 1:
                    # debug: dump g2 to out and skip exchange
                    nrow2 = min(GSZ, RS - g * GSZ)
                    nc.sync.dma_start(t_out[g * GSZ:g * GSZ + nrow2, :],
                                      g2loc[:nrow2, g, :])
                if PHASE < 2:
                    return
                if (g % GPC == GPC - 1 or g == GP - 1) and not ag_emitted[k]:
                    ag_emitted[k] = True
                    w = k // CPW
                    base = NC * AGL * k - w * WIN
                    nc.gpsimd.collective_compute(
                        "AllGather", mybir.AluOpType.bypass,
                        replica_groups=[list(range(NC))],
                        ins=[g2in[k][:, :]],
                        outs=[g2win[w][base:base + NC * lens[k], :]])

            run_phase(L1, t_idx1, dst1_t, coef1_t, FIN,
                      [t_x[w * WIN:w * WIN + winlen[w], :] for w in range(NW)],
                      post_l1, "1")

            # ---------------- phase B: layer 2
            def post_l2(g, agg, wp_, pp_):
                t1 = wp_.tile([128, FOUT], f32, tag="t1")
                nc.vector.tensor_tensor(out=t1[:], in0=agg[:],
                                        in1=g2loc[:, g, :],
                                        op=mybir.AluOpType.add)
                t2 = wp_.tile([128, FOUT], f32, tag="t2")
                nc.vector.tensor_scalar(out=t2[:], in0=t1[:],
                                        scalar1=dinv_t[:, g:g + 1],
                                        scalar2=None,
                                        op0=mybir.AluOpType.mult)
                t3 = wp_.tile([128, FOUT], f32, tag="t3")
                nc.vector.tensor_tensor(out=t3[:], in0=t2[:], in1=b2_t[:],
                                        op=mybir.AluOpType.add)
                nrow = min(GSZ, RS - g * GSZ)
                nc.sync.dma_start(t_out[g * GSZ:g * GSZ + nrow, :],
                                  t3[:nrow, :])

            if PHASE >= 3:
                run_phase(L2, t_idx2, dst2_t, None, FOUT,
                          [g2win[w][:, :] for w in range(NW)],
                          post_l2, "2")
            elif PHASE == 2:
                # drain AG outputs into out so the program has output writers
                for g in range(GP):
                    tdr = cp.tile([128, FOUT], f32, name="tdr", tag="tdr")
                    w = (g * GSZ) // WIN
                    nc.sync.dma_start(tdr[:], g2win[0][0:128, :])
                    nrow = min(GSZ, RS - g * GSZ)
                    nc.sync.dma_start(t_out[g * GSZ:g * GSZ + nrow, :],
                                      tdr[:nrow, :])

    nc.compile()
    return nc


def _run(inputs, act="gelu", trace=False, use_sim=False, trace_kwargs=None):
    x = np.ascontiguousarray(np.asarray(inputs["x"], np.float32))
    key = (hash(np.asarray(inputs["edge_index"]).tobytes()), act)
    if key not in _cache:
        pp = _preprocess(x, np.asarray(inputs["edge_index"]),
                         inputs["W1"], inputs["b1"], inputs["W2"],
                         inputs["b2"])
        nc = _build_nc(pp, act=act)
        _cache.clear()
        _cache[key] = (pp, nc)
    pp, nc = _cache[key]

    in_maps = []
    for c in range(NC):
        in_maps.append({
            "x": x,
            "xs": pp["xs_pc"][c],
            "idx1": pp["L1"]["idx"][c],
            "idx2": pp["L2"]["idx"][c],
            "dst1": pp["L1"]["dst"][c],
            "coef1": pp["L1"]["coef"][c],
            "dst2": pp["L2"]["dst"][c],
            "dinv_pc": pp["dinv_pc"][c],
            "dinv2_pc": pp["dinv2_pc"][c],
            "W1": pp["W1"], "W2": pp["W2"],
            "b1bc": pp["b1bc"], "b2bc": pp["b2bc"],
        })
    if use_sim:
        from concourse.bass_interp import MultiCoreSim
        sim = MultiCoreSim(nc, num_cores=NC)
        for ci, core in sim.cores.items():
            for k, v in in_maps[ci].items():
                core.tensor(k)[:] = v
        sim.simulate()
        outs = [np.array(core.tensor("out"))
                for _, core in sorted(sim.cores.items())]
        return np.concatenate(outs, 0), None
    from concourse.bass_utils import run_bass_kernel_spmd
    res = run_bass_kernel_spmd(nc, in_maps, core_ids=list(range(NC)),
                               trace=trace, **(trace_kwargs or {}))
    out = np.concatenate([res.results[c]["out"] for c in range(NC)], 0)
    return out, res


def kernel(**inputs) -> np.ndarray:
    out, _ = _run(inputs)
    return out


def bench(inputs, act="gelu", iters=8):
    """Measure per-execution device time by chaining `iters` executions of
    the NEFF inside one jit (outputs feed the next iteration's output
    operands, defeating CSE) and comparing against a 1-iteration call."""
    import time
    import jax
    from jax.sharding import Mesh, PartitionSpec
    from jax.experimental.shard_map import shard_map
    from concourse import mybir
    from concourse import bass2jax as b2j

    x = np.ascontiguousarray(np.asarray(inputs["x"], np.float32))
    key = (hash(np.asarray(inputs["edge_index"]).tobytes()), act)
    if key not in _cache:
        _run(inputs, act=act)   # build + correctness path
    pp, nc = _cache[key]
    b2j.install_neuronx_cc_hook()

    in_maps = []
    for c in range(NC):
        in_maps.append({
            "x": x, "xs": pp["xs_pc"][c],
            "idx1": pp["L1"]["idx"][c], "idx2": pp["L2"]["idx"][c],
            "dst1": pp["L1"]["dst"][c], "coef1": pp["L1"]["coef"][c],
            "dst2": pp["L2"]["dst"][c],
            "dinv_pc": pp["dinv_pc"][c], "dinv2_pc": pp["dinv2_pc"][c],
            "W1": pp["W1"], "W2": pp["W2"],
            "b1bc": pp["b1bc"], "b2bc": pp["b2bc"],
        })

    in_names, out_names, out_avals, zero_outs = [], [], [], []
    import concourse.mybir as mb
    pid_name = (nc.partition_id_tensor.name
                if nc.partition_id_tensor is not None else None)
    for alloc in nc.m.functions[0].allocations:
        if not isinstance(alloc, mb.MemoryLocationSet):
            continue
        name = alloc.memorylocations[0].name
        if alloc.kind == "ExternalInput":
            if name == pid_name:
                continue
            in_names.append(name)
        elif alloc.kind == "ExternalOutput":
            out_names.append(name)
            shape = tuple(alloc.tensor_shape)
            dtype = mb.dt.np(alloc.dtype)
            out_avals.append(jax.core.ShapedArray(shape, dtype))
            zero_outs.append(np.zeros(shape, dtype))
    n_params = len(in_names)
    all_names = in_names + out_names
    if pid_name is not None:
        all_names = all_names + [pid_name]

    def one_call(params, outs_in):
        extra = ([b2j.partition_id_tensor()] if pid_name is not None else [])
        outs = b2j._bass_exec_p.bind(
            *params, *outs_in, *extra,
            out_avals=tuple(out_avals),
            in_names=tuple(all_names),
            out_names=tuple(out_names),
            lowering_input_output_aliases=(),
            sim_require_finite=True,
            sim_require_nnan=True,
            nc=nc,
        )
        return list(outs)

    def _body(*args):
        params = list(args[:n_params])
        outs = list(args[n_params:])
        outs = one_call(params, outs)
        return tuple(outs)

    devices = jax.devices()[:NC]
    mesh = Mesh(np.asarray(devices), ("core",))
    specs = (PartitionSpec("core"),)
    per_core = [[np.asarray(m[nm]) for nm in in_names] for m in in_maps]
    concat_in = [np.concatenate([per_core[c][i] for c in range(NC)], 0)
                 for i in range(n_params)]
    concat_zeros = [np.zeros((NC * z.shape[0], *z.shape[1:]), z.dtype)
                    for z in zero_outs]

    nin = n_params + len(out_names)
    fn = jax.jit(shard_map(_body, mesh=mesh,
                           in_specs=specs * nin,
                           out_specs=specs * len(out_names),
                           check_rep=False),
                 donate_argnums=tuple(range(n_params, nin)))
    from jax.sharding import NamedSharding
    shard = NamedSharding(mesh, PartitionSpec("core"))
    dev_in = [jax.device_put(a, shard) for a in concat_in]
    outs = [jax.device_put(a, shard) for a in concat_zeros]
    outs = fn(*dev_in, *outs)          # warm: compile + first exec
    jax.block_until_ready(outs)

    results = {}
    for k in (1, iters):
        best = None
        for _ in range(3):
            t0 = time.perf_counter()
            o = outs
            for _ in range(k):
                o = fn(*dev_in, *o)
            jax.block_until_ready(o)
            dt = time.perf_counter() - t0
            outs = o
            best = dt if best is None else min(best, dt)
        results[k] = best
    per_iter_ns = (results[iters] - results[1]) / (iters - 1) * 1e9
    return per_iter_ns, results

